# revision 4
# baseline (speedup 1.0000x reference)
"""Segment-gather-mean kernel for Trainium2 (8 NeuronCores).

out[a] = mean over edges e with ancestors[e] == a of features[curr_nodes_idx[e]]

Strategy: shard the 50000 output segments across 8 cores (6250 each). The host
buckets each core's edges by 128-segment window, sorted so each window owns a
fixed number of 128-edge chunks. On device, each chunk's rows are fetched with
dma_gather (one row per partition), a one-hot [edge, seg] matrix is built on the
vector engine by comparing an iota row against each edge's local segment id, and
a PE matmul accumulates onehot.T @ gathered_rows into a per-window PSUM tile.
The scalar engine applies the host-precomputed 1/count scale during the
PSUM->SBUF flush, and the window is DMA'd to the output rows.

dma_gather indices are int16, so edges are split per window into chunks whose
node index is < 32768 (gathered from the table base) and >= 32768 (gathered
from a base offset 32768 rows in). Pad slots use node 0 with segment id -1,
which produces an all-zero one-hot column and contributes nothing.
"""

import math
import os
import sys

sys.path.insert(0, "/opt/trn_rl_repo")

import numpy as np

# set KERNEL_TRACE=1 to capture an NTFF profile; the BassKernelResults of the
# last run (with .exec_time_ns) is stored in `last_results`.
TRACE = os.environ.get("KERNEL_TRACE", "0") == "1"
last_results = None

P = 128
D = 128
N_OUT = 50000
N_NODES = 50000
N_CORES = 8
NSEG = N_OUT // N_CORES          # segments per core
W = math.ceil(NSEG / P)          # seg windows per core
LAST_ROWS = NSEG - (W - 1) * P   # rows in the final window
SPLIT = 32768                    # int16 gather index limit
G = 4                            # windows per gather group

_nc_cache = {}


def _build_nc(CA, CB, t_iters=1):
    import concourse.bacc as bacc
    import concourse.mybir as mybir
    from concourse.tile import TileContext

    C = CA + CB
    nc = bacc.Bacc("TRN2", target_bir_lowering=False, debug=False,
                   num_devices=N_CORES)
    feat = nc.dram_tensor("feat", [N_NODES, D], mybir.dt.float32,
                          kind="ExternalInput")
    idxa = nc.dram_tensor("idxa", [P, W * CA * 8], mybir.dt.int16,
                          kind="ExternalInput")
    idxb = nc.dram_tensor("idxb", [P, W * CB * 8], mybir.dt.int16,
                          kind="ExternalInput")
    seg = nc.dram_tensor("seg", [P, W * C], mybir.dt.float32,
                         kind="ExternalInput")
    recip = nc.dram_tensor("recip", [P, W], mybir.dt.float32,
                           kind="ExternalInput")
    iota = nc.dram_tensor("iota", [P, P], mybir.dt.float32,
                          kind="ExternalInput")
    out = nc.dram_tensor("out", [NSEG, D], mybir.dt.float32,
                         kind="ExternalOutput")

    n_grp = math.ceil(W / G)

    with TileContext(nc) as tc:
        with (
            tc.tile_pool(name="const", bufs=1) as cpool,
            tc.tile_pool(name="gatha", bufs=2) as gapool,
            tc.tile_pool(name="gathb", bufs=2) as gbpool,
            tc.tile_pool(name="oh", bufs=4) as ohpool,
            tc.tile_pool(name="psum", bufs=4, space="PSUM") as ppool,
            tc.tile_pool(name="osb", bufs=4) as opool,
        ):
            idxa_sb = cpool.tile([P, W * CA * 8], mybir.dt.int16)
            idxb_sb = cpool.tile([P, W * CB * 8], mybir.dt.int16)
            seg_sb = cpool.tile([P, W * C], mybir.dt.float32)
            recip_sb = cpool.tile([P, W], mybir.dt.float32)
            iota_sb = cpool.tile([P, P], mybir.dt.float32)
            nc.sync.dma_start(idxa_sb[:], idxa[:])
            nc.sync.dma_start(idxb_sb[:], idxb[:])
            nc.sync.dma_start(seg_sb[:], seg[:])
            nc.sync.dma_start(recip_sb[:], recip[:])
            nc.sync.dma_start(iota_sb[:], iota[:])

            def body(_=None):
                for grp in range(n_grp):
                    wlo = grp * G
                    gw = min(W, wlo + G) - wlo
                    ga = gapool.tile([P, G * CA, D], mybir.dt.float32)
                    gb = gbpool.tile([P, G * CB, D], mybir.dt.float32)
                    na = gw * CA * P
                    nb = gw * CB * P
                    nc.gpsimd.dma_gather(
                        ga[:, : gw * CA, :], feat[:SPLIT, :],
                        idxa_sb[:, wlo * CA * 8:(wlo + gw) * CA * 8],
                        na, na, D, single_packet=False)
                    nc.gpsimd.dma_gather(
                        gb[:, : gw * CB, :], feat[SPLIT:, :],
                        idxb_sb[:, wlo * CB * 8:(wlo + gw) * CB * 8],
                        nb, nb, D, single_packet=False)
                    for wi in range(gw):
                        w = wlo + wi
                        ps = ppool.tile([P, D], mybir.dt.float32, space="PSUM")
                        for j in range(C):
                            oh = ohpool.tile([P, P], mybir.dt.float32)
                            nc.vector.tensor_scalar(
                                out=oh[:],
                                in0=iota_sb[:],
                                scalar1=seg_sb[:, w * C + j: w * C + j + 1],
                                scalar2=None,
                                op0=mybir.AluOpType.is_equal,
                            )
                            if j < CA:
                                rhs = ga[:, wi * CA + (j), :]
                            else:
                                rhs = gb[:, wi * CB + (j - CA), :]
                            nc.tensor.matmul(
                                ps[:], lhsT=oh[:], rhs=rhs,
                                start=(j == 0), stop=(j == C - 1))
                        osb = opool.tile([P, D], mybir.dt.float32)
                        nc.scalar.activation(
                            osb[:], ps[:], mybir.ActivationFunctionType.Copy,
                            scale=recip_sb[:, w:w + 1])
                        rows = P if w < W - 1 else LAST_ROWS
                        nc.sync.dma_start(out[w * P: w * P + rows, :],
                                          osb[:rows, :])

            if t_iters == 1:
                body()
            else:
                with tc.For_i(0, t_iters, 1) as _i:
                    body()

    nc.compile()
    return nc


def _prep_core(anc_l, nodes_l, CA, CB):
    """Build idxa/idxb/seg host arrays for one core.

    anc_l: local ancestor ids in [0, NSEG); nodes_l: node ids in [0, N_NODES).
    """
    C = CA + CB
    w_all = anc_l // P
    s_all = (anc_l % P).astype(np.float32)
    low = nodes_l < SPLIT

    idxA = np.zeros((W, CA * P), np.int16)
    segA = np.full((W, CA * P), -1.0, np.float32)
    idxB = np.zeros((W, CB * P), np.int16)
    segB = np.full((W, CB * P), -1.0, np.float32)

    for sel, idx_arr, seg_arr, off in (
        (low, idxA, segA, 0),
        (~low, idxB, segB, SPLIT),
    ):
        wsel = w_all[sel]
        nsel = nodes_l[sel] - off
        ssel = s_all[sel]
        order = np.argsort(wsel, kind="stable")
        wsel = wsel[order]
        nsel = nsel[order]
        ssel = ssel[order]
        cnt = np.bincount(wsel, minlength=W)
        offs = np.concatenate([[0], np.cumsum(cnt)])
        for w in range(W):
            k = cnt[w]
            idx_arr[w, :k] = nsel[offs[w]: offs[w] + k].astype(np.int16)
            seg_arr[w, :k] = ssel[offs[w]: offs[w] + k]

    def wrap_idx(arr, CX):
        # idx i of each window -> [i % 16, i // 16]; windows along columns.
        a = arr.reshape(W, CX * 8, 16).swapaxes(1, 2)   # [W, 16, CX*8]
        a = a.transpose(1, 0, 2).reshape(16, W * CX * 8)
        return np.tile(a, (8, 1))                        # [128, W*CX*8]

    segAB = np.concatenate(
        [segA.reshape(W, CA, P), segB.reshape(W, CB, P)], axis=1)
    seg_sb = np.ascontiguousarray(segAB.transpose(2, 0, 1).reshape(P, W * C))
    return wrap_idx(idxA, CA), wrap_idx(idxB, CB), seg_sb


def _prepare(features, nodes, anc):
    """Host-side sharding: returns (CA, CB, in_maps)."""
    core = anc // NSEG
    anc_local = anc - core * NSEG

    # global chunk counts (same NEFF for every core)
    maxA = 0
    maxB = 0
    per_core = []
    for c in range(N_CORES):
        m = core == c
        a_l = anc_local[m]
        n_l = nodes[m]
        per_core.append((a_l, n_l))
        w_l = a_l // P
        low = n_l < SPLIT
        ca = np.bincount(w_l[low], minlength=W).max() if low.any() else 0
        cb = np.bincount(w_l[~low], minlength=W).max() if (~low).any() else 0
        maxA = max(maxA, int(ca))
        maxB = max(maxB, int(cb))
    CA = max(1, math.ceil(maxA / P))
    CB = max(1, math.ceil(maxB / P))

    cnt = np.bincount(anc, minlength=N_OUT).astype(np.float32)
    recip_all = (1.0 / np.maximum(cnt, 1.0)).astype(np.float32)

    iota = np.ascontiguousarray(
        np.tile(np.arange(P, dtype=np.float32)[None, :], (P, 1)))

    in_maps = []
    for c in range(N_CORES):
        a_l, n_l = per_core[c]
        ia, ib, sg = _prep_core(a_l, n_l, CA, CB)
        r = recip_all[c * NSEG:(c + 1) * NSEG]
        r = np.concatenate([r, np.ones(W * P - NSEG, np.float32)])
        r_sb = np.ascontiguousarray(r.reshape(W, P).T)
        in_maps.append({
            "feat": features,
            "idxa": ia,
            "idxb": ib,
            "seg": sg,
            "recip": r_sb,
            "iota": iota,
        })
    return CA, CB, in_maps


def kernel(**inputs):
    from concourse.bass_utils import run_bass_kernel_spmd

    features = np.ascontiguousarray(
        np.asarray(inputs["features"], dtype=np.float32))
    nodes = np.asarray(inputs["curr_nodes_idx"]).astype(np.int64)
    anc = np.asarray(inputs["ancestors"]).astype(np.int64)
    uall = np.asarray(inputs["uall_ancestors_idx"]).astype(np.int64)

    CA, CB, in_maps = _prepare(features, nodes, anc)

    key = (CA, CB)
    if key not in _nc_cache:
        _nc_cache[key] = _build_nc(CA, CB)
    nc = _nc_cache[key]

    res = run_bass_kernel_spmd(nc, in_maps, core_ids=list(range(N_CORES)),
                               trace=TRACE)
    global last_results
    last_results = res
    mean = np.concatenate([res.results[c]["out"] for c in range(N_CORES)],
                          axis=0)
    out = np.zeros((N_OUT, D), np.float32)
    out[uall] = mean
    return out


# revision 6
# speedup vs baseline: 1.6490x; 1.6490x over previous
"""Segment-gather-mean kernel for Trainium2 (8 NeuronCores).

out[a] = mean over edges e with ancestors[e] == a of features[curr_nodes_idx[e]]

Strategy: shard the 50000 output segments across 8 cores (6250 each). The host
buckets each core's edges by 128-segment window, sorted so each window owns a
fixed number of 128-edge chunks. On device, each chunk's rows are fetched with
dma_gather (one row per partition), a one-hot [edge, seg] matrix is built on the
vector engine by comparing an iota row against each edge's local segment id, and
a PE matmul accumulates onehot.T @ gathered_rows into a per-window PSUM tile.
The scalar engine applies the host-precomputed 1/count scale during the
PSUM->SBUF flush, and the window is DMA'd to the output rows.

dma_gather indices are int16, so edges are split per window into chunks whose
node index is < 32768 (gathered from the table base) and >= 32768 (gathered
from a base offset 32768 rows in). Pad slots use node 0 with segment id -1,
which produces an all-zero one-hot column and contributes nothing.
"""

import math
import os
import sys

sys.path.insert(0, "/opt/trn_rl_repo")

import numpy as np

# set KERNEL_TRACE=1 to capture an NTFF profile; the BassKernelResults of the
# last run (with .exec_time_ns) is stored in `last_results`.
TRACE = os.environ.get("KERNEL_TRACE", "0") == "1"
last_results = None

P = 128
D = 128
N_OUT = 50000
N_NODES = 50000
N_CORES = 8
NSEG = N_OUT // N_CORES          # segments per core
W = math.ceil(NSEG / P)          # seg windows per core
LAST_ROWS = NSEG - (W - 1) * P   # rows in the final window
SPLIT = 32768                    # int16 gather index limit
G = 4                            # windows per gather group

_nc_cache = {}


def _build_nc(CA, CB, t_iters=1):
    import concourse.bacc as bacc
    import concourse.mybir as mybir
    from concourse.tile import TileContext

    C = CA + CB
    nc = bacc.Bacc("TRN2", target_bir_lowering=False, debug=False,
                   num_devices=N_CORES, num_swdge_queues=4)
    feat = nc.dram_tensor("feat", [N_NODES, D], mybir.dt.float32,
                          kind="ExternalInput")
    idxa = nc.dram_tensor("idxa", [P, W * CA * 8], mybir.dt.int16,
                          kind="ExternalInput")
    idxb = nc.dram_tensor("idxb", [P, W * CB * 8], mybir.dt.int16,
                          kind="ExternalInput")
    seg = nc.dram_tensor("seg", [P, W * C], mybir.dt.float32,
                         kind="ExternalInput")
    recip = nc.dram_tensor("recip", [P, W], mybir.dt.float32,
                           kind="ExternalInput")
    iota = nc.dram_tensor("iota", [P, P], mybir.dt.float32,
                          kind="ExternalInput")
    out = nc.dram_tensor("out", [NSEG, D], mybir.dt.float32,
                         kind="ExternalOutput")

    n_grp = math.ceil(W / G)

    with TileContext(nc) as tc:
        with (
            tc.tile_pool(name="const", bufs=1) as cpool,
            tc.tile_pool(name="gatha", bufs=2) as gapool,
            tc.tile_pool(name="gathb", bufs=2) as gbpool,
            tc.tile_pool(name="oh", bufs=4) as ohpool,
            tc.tile_pool(name="psum", bufs=4, space="PSUM") as ppool,
            tc.tile_pool(name="osb", bufs=4) as opool,
        ):
            idxa_sb = cpool.tile([P, W * CA * 8], mybir.dt.int16)
            idxb_sb = cpool.tile([P, W * CB * 8], mybir.dt.int16)
            seg_sb = cpool.tile([P, W * C], mybir.dt.float32)
            recip_sb = cpool.tile([P, W], mybir.dt.float32)
            iota_sb = cpool.tile([P, P], mybir.dt.float32)
            nc.sync.dma_start(idxa_sb[:], idxa[:])
            nc.sync.dma_start(idxb_sb[:], idxb[:])
            nc.sync.dma_start(seg_sb[:], seg[:])
            nc.sync.dma_start(recip_sb[:], recip[:])
            nc.sync.dma_start(iota_sb[:], iota[:])

            def body(_=None):
                for grp in range(n_grp):
                    wlo = grp * G
                    gw = min(W, wlo + G) - wlo
                    ga = gapool.tile([P, G * CA, D], mybir.dt.float32)
                    gb = gbpool.tile([P, G * CB, D], mybir.dt.float32)
                    na = gw * CA * P
                    nb = gw * CB * P
                    nc.gpsimd.dma_gather(
                        ga[:, : gw * CA, :], feat[:SPLIT, :],
                        idxa_sb[:, wlo * CA * 8:(wlo + gw) * CA * 8],
                        na, na, D, single_packet=False,
                        queue_num=(2 * grp) % 4)
                    nc.gpsimd.dma_gather(
                        gb[:, : gw * CB, :], feat[SPLIT:, :],
                        idxb_sb[:, wlo * CB * 8:(wlo + gw) * CB * 8],
                        nb, nb, D, single_packet=False,
                        queue_num=(2 * grp + 1) % 4)
                    for wi in range(gw):
                        w = wlo + wi
                        ps = ppool.tile([P, D], mybir.dt.float32, space="PSUM")
                        for j in range(C):
                            oh = ohpool.tile([P, P], mybir.dt.float32)
                            nc.vector.tensor_scalar(
                                out=oh[:],
                                in0=iota_sb[:],
                                scalar1=seg_sb[:, w * C + j: w * C + j + 1],
                                scalar2=None,
                                op0=mybir.AluOpType.is_equal,
                            )
                            if j < CA:
                                rhs = ga[:, wi * CA + (j), :]
                            else:
                                rhs = gb[:, wi * CB + (j - CA), :]
                            nc.tensor.matmul(
                                ps[:], lhsT=oh[:], rhs=rhs,
                                start=(j == 0), stop=(j == C - 1))
                        osb = opool.tile([P, D], mybir.dt.float32)
                        nc.scalar.activation(
                            osb[:], ps[:], mybir.ActivationFunctionType.Copy,
                            scale=recip_sb[:, w:w + 1])
                        rows = P if w < W - 1 else LAST_ROWS
                        nc.sync.dma_start(out[w * P: w * P + rows, :],
                                          osb[:rows, :])

            if t_iters == 1:
                body()
            else:
                with tc.For_i(0, t_iters, 1) as _i:
                    body()

    nc.compile()
    return nc


def _prep_core(anc_l, nodes_l, CA, CB):
    """Build idxa/idxb/seg host arrays for one core.

    anc_l: local ancestor ids in [0, NSEG); nodes_l: node ids in [0, N_NODES).
    """
    C = CA + CB
    w_all = anc_l // P
    s_all = (anc_l % P).astype(np.float32)
    low = nodes_l < SPLIT

    idxA = np.zeros((W, CA * P), np.int16)
    segA = np.full((W, CA * P), -1.0, np.float32)
    idxB = np.zeros((W, CB * P), np.int16)
    segB = np.full((W, CB * P), -1.0, np.float32)

    for sel, idx_arr, seg_arr, off in (
        (low, idxA, segA, 0),
        (~low, idxB, segB, SPLIT),
    ):
        wsel = w_all[sel]
        nsel = nodes_l[sel] - off
        ssel = s_all[sel]
        order = np.argsort(wsel, kind="stable")
        wsel = wsel[order]
        nsel = nsel[order]
        ssel = ssel[order]
        cnt = np.bincount(wsel, minlength=W)
        offs = np.concatenate([[0], np.cumsum(cnt)])
        for w in range(W):
            k = cnt[w]
            idx_arr[w, :k] = nsel[offs[w]: offs[w] + k].astype(np.int16)
            seg_arr[w, :k] = ssel[offs[w]: offs[w] + k]

    def wrap_idx(arr, CX):
        # idx i of each window -> [i % 16, i // 16]; windows along columns.
        a = arr.reshape(W, CX * 8, 16).swapaxes(1, 2)   # [W, 16, CX*8]
        a = a.transpose(1, 0, 2).reshape(16, W * CX * 8)
        return np.tile(a, (8, 1))                        # [128, W*CX*8]

    segAB = np.concatenate(
        [segA.reshape(W, CA, P), segB.reshape(W, CB, P)], axis=1)
    seg_sb = np.ascontiguousarray(segAB.transpose(2, 0, 1).reshape(P, W * C))
    return wrap_idx(idxA, CA), wrap_idx(idxB, CB), seg_sb


def _prepare(features, nodes, anc):
    """Host-side sharding: returns (CA, CB, in_maps)."""
    core = anc // NSEG
    anc_local = anc - core * NSEG

    # global chunk counts (same NEFF for every core)
    maxA = 0
    maxB = 0
    per_core = []
    for c in range(N_CORES):
        m = core == c
        a_l = anc_local[m]
        n_l = nodes[m]
        per_core.append((a_l, n_l))
        w_l = a_l // P
        low = n_l < SPLIT
        ca = np.bincount(w_l[low], minlength=W).max() if low.any() else 0
        cb = np.bincount(w_l[~low], minlength=W).max() if (~low).any() else 0
        maxA = max(maxA, int(ca))
        maxB = max(maxB, int(cb))
    CA = max(1, math.ceil(maxA / P))
    CB = max(1, math.ceil(maxB / P))

    cnt = np.bincount(anc, minlength=N_OUT).astype(np.float32)
    recip_all = (1.0 / np.maximum(cnt, 1.0)).astype(np.float32)

    iota = np.ascontiguousarray(
        np.tile(np.arange(P, dtype=np.float32)[None, :], (P, 1)))

    in_maps = []
    for c in range(N_CORES):
        a_l, n_l = per_core[c]
        ia, ib, sg = _prep_core(a_l, n_l, CA, CB)
        r = recip_all[c * NSEG:(c + 1) * NSEG]
        r = np.concatenate([r, np.ones(W * P - NSEG, np.float32)])
        r_sb = np.ascontiguousarray(r.reshape(W, P).T)
        in_maps.append({
            "feat": features,
            "idxa": ia,
            "idxb": ib,
            "seg": sg,
            "recip": r_sb,
            "iota": iota,
        })
    return CA, CB, in_maps


def kernel(**inputs):
    from concourse.bass_utils import run_bass_kernel_spmd

    features = np.ascontiguousarray(
        np.asarray(inputs["features"], dtype=np.float32))
    nodes = np.asarray(inputs["curr_nodes_idx"]).astype(np.int64)
    anc = np.asarray(inputs["ancestors"]).astype(np.int64)
    uall = np.asarray(inputs["uall_ancestors_idx"]).astype(np.int64)

    CA, CB, in_maps = _prepare(features, nodes, anc)

    key = (CA, CB)
    if key not in _nc_cache:
        _nc_cache[key] = _build_nc(CA, CB)
    nc = _nc_cache[key]

    res = run_bass_kernel_spmd(nc, in_maps, core_ids=list(range(N_CORES)),
                               trace=TRACE)
    global last_results
    last_results = res
    mean = np.concatenate([res.results[c]["out"] for c in range(N_CORES)],
                          axis=0)
    out = np.zeros((N_OUT, D), np.float32)
    out[uall] = mean
    return out


# revision 12
# speedup vs baseline: 2.8214x; 1.7110x over previous
"""Segment-gather-mean kernel for Trainium2 (8 NeuronCores).

out[a] = mean over edges e with ancestors[e] == a of features[curr_nodes_idx[e]]

Strategy: shard the 50000 output segments across 8 cores (6250 each). The host
buckets each core's edges by 128-segment window, sorted so each window owns a
fixed number of 128-edge chunks. On device, each chunk's rows are fetched with
dma_gather (one row per partition), a one-hot [edge, seg] matrix is built on the
vector engine by comparing an iota row against each edge's local segment id, and
a PE matmul accumulates onehot.T @ gathered_rows into a per-window PSUM tile.
The scalar engine applies the host-precomputed 1/count scale during the
PSUM->SBUF flush, and the window is DMA'd to the output rows.

dma_gather indices are int16, so edges are split per window into chunks whose
node index is < 32768 (gathered from the table base) and >= 32768 (gathered
from a base offset 32768 rows in). Pad slots use node 0 with segment id -1,
which produces an all-zero one-hot column and contributes nothing.
"""

import math
import os
import sys

sys.path.insert(0, "/opt/trn_rl_repo")

import numpy as np

# set KERNEL_TRACE=1 to capture an NTFF profile; the BassKernelResults of the
# last run (with .exec_time_ns) is stored in `last_results`.
TRACE = os.environ.get("KERNEL_TRACE", "0") == "1"
last_results = None

P = 128
D = 128
N_OUT = 50000
N_NODES = 50000
N_CORES = 8
NSEG = N_OUT // N_CORES          # segments per core
W = math.ceil(NSEG / P)          # seg windows per core
LAST_ROWS = NSEG - (W - 1) * P   # rows in the final window
SPLIT = 32768                    # int16 gather index limit
G = 2                            # windows per gather group

_nc_cache = {}


def _build_nc(CA, CB, t_iters=1):
    import concourse.bacc as bacc
    import concourse.mybir as mybir
    from concourse.tile import TileContext

    C = CA + CB
    nc = bacc.Bacc("TRN2", target_bir_lowering=False, debug=False,
                   num_devices=N_CORES, num_swdge_queues=4)
    feat = nc.dram_tensor("feat", [N_NODES, D], mybir.dt.float32,
                          kind="ExternalInput")
    idxa = nc.dram_tensor("idxa", [P, W * CA * 8], mybir.dt.int16,
                          kind="ExternalInput")
    idxb = nc.dram_tensor("idxb", [P, W * CB * 8], mybir.dt.int16,
                          kind="ExternalInput")
    seg = nc.dram_tensor("seg", [P, W * C], mybir.dt.float32,
                         kind="ExternalInput")
    recip = nc.dram_tensor("recip", [P, W], mybir.dt.float32,
                           kind="ExternalInput")
    iota = nc.dram_tensor("iota", [P, C, P], mybir.dt.float32,
                          kind="ExternalInput")
    out = nc.dram_tensor("out", [NSEG, D], mybir.dt.float32,
                         kind="ExternalOutput")

    n_grp = math.ceil(W / G)

    with TileContext(nc) as tc:
        with (
            tc.tile_pool(name="const", bufs=1) as cpool,
            tc.tile_pool(name="gatha", bufs=4) as gapool,
            tc.tile_pool(name="gathb", bufs=4) as gbpool,
            tc.tile_pool(name="oh", bufs=3) as ohpool,
            tc.tile_pool(name="psum", bufs=4, space="PSUM") as ppool,
            tc.tile_pool(name="osb", bufs=4) as opool,
        ):
            idxa_sb = cpool.tile([P, W * CA * 8], mybir.dt.int16)
            idxb_sb = cpool.tile([P, W * CB * 8], mybir.dt.int16)
            seg_sb = cpool.tile([P, W * C], mybir.dt.float32)
            recip_sb = cpool.tile([P, W], mybir.dt.float32)
            iota_sb = cpool.tile([P, C, P], mybir.dt.float32)
            nc.sync.dma_start(idxa_sb[:], idxa[:])
            nc.sync.dma_start(idxb_sb[:], idxb[:])
            nc.sync.dma_start(seg_sb[:], seg[:])
            nc.sync.dma_start(recip_sb[:], recip[:])
            nc.sync.dma_start(iota_sb[:], iota[:])

            def body(_=None):
                for grp in range(n_grp):
                    wlo = grp * G
                    gw = min(W, wlo + G) - wlo
                    ga = gapool.tile([P, G * CA, D], mybir.dt.float32)
                    gb = gbpool.tile([P, G * CB, D], mybir.dt.float32)
                    na = gw * CA * P
                    nb = gw * CB * P
                    nc.gpsimd.dma_gather(
                        ga[:, : gw * CA, :], feat[:SPLIT, :],
                        idxa_sb[:, wlo * CA * 8:(wlo + gw) * CA * 8],
                        na, na, D, single_packet=False,
                        queue_num=(2 * grp) % 4)
                    nc.gpsimd.dma_gather(
                        gb[:, : gw * CB, :], feat[SPLIT:, :],
                        idxb_sb[:, wlo * CB * 8:(wlo + gw) * CB * 8],
                        nb, nb, D, single_packet=False,
                        queue_num=(2 * grp + 1) % 4)
                    for wi in range(gw):
                        w = wlo + wi
                        oh = ohpool.tile([P, C, P], mybir.dt.float32)
                        nc.vector.tensor_tensor(
                            out=oh[:],
                            in0=iota_sb[:],
                            in1=seg_sb[:, w * C:(w + 1) * C].to_broadcast(
                                [P, C, P]),
                            op=mybir.AluOpType.is_equal,
                        )
                        ps = ppool.tile([P, D], mybir.dt.float32, space="PSUM")
                        for j in range(C):
                            if j < CA:
                                rhs = ga[:, wi * CA + (j), :]
                            else:
                                rhs = gb[:, wi * CB + (j - CA), :]
                            nc.tensor.matmul(
                                ps[:], lhsT=oh[:, j, :], rhs=rhs,
                                start=(j == 0), stop=(j == C - 1))
                        osb = opool.tile([P, D], mybir.dt.float32)
                        nc.scalar.activation(
                            osb[:], ps[:], mybir.ActivationFunctionType.Copy,
                            scale=recip_sb[:, w:w + 1])
                        rows = P if w < W - 1 else LAST_ROWS
                        nc.sync.dma_start(out[w * P: w * P + rows, :],
                                          osb[:rows, :])

            if t_iters == 1:
                body()
            else:
                with tc.For_i(0, t_iters, 1) as _i:
                    body()

    nc.compile()
    return nc


def _prep_core(anc_l, nodes_l, CA, CB):
    """Build idxa/idxb/seg host arrays for one core.

    anc_l: local ancestor ids in [0, NSEG); nodes_l: node ids in [0, N_NODES).
    """
    C = CA + CB
    w_all = anc_l // P
    s_all = (anc_l % P).astype(np.float32)
    low = nodes_l < SPLIT

    idxA = np.zeros((W, CA * P), np.int16)
    segA = np.full((W, CA * P), -1.0, np.float32)
    idxB = np.zeros((W, CB * P), np.int16)
    segB = np.full((W, CB * P), -1.0, np.float32)

    for sel, idx_arr, seg_arr, off in (
        (low, idxA, segA, 0),
        (~low, idxB, segB, SPLIT),
    ):
        wsel = w_all[sel]
        nsel = nodes_l[sel] - off
        ssel = s_all[sel]
        order = np.argsort(wsel, kind="stable")
        wsel = wsel[order]
        nsel = nsel[order]
        ssel = ssel[order]
        cnt = np.bincount(wsel, minlength=W)
        offs = np.concatenate([[0], np.cumsum(cnt)])
        for w in range(W):
            k = cnt[w]
            idx_arr[w, :k] = nsel[offs[w]: offs[w] + k].astype(np.int16)
            seg_arr[w, :k] = ssel[offs[w]: offs[w] + k]

    def wrap_idx(arr, CX):
        # idx i of each window -> [i % 16, i // 16]; windows along columns.
        a = arr.reshape(W, CX * 8, 16).swapaxes(1, 2)   # [W, 16, CX*8]
        a = a.transpose(1, 0, 2).reshape(16, W * CX * 8)
        return np.tile(a, (8, 1))                        # [128, W*CX*8]

    segAB = np.concatenate(
        [segA.reshape(W, CA, P), segB.reshape(W, CB, P)], axis=1)
    seg_sb = np.ascontiguousarray(segAB.transpose(2, 0, 1).reshape(P, W * C))
    return wrap_idx(idxA, CA), wrap_idx(idxB, CB), seg_sb


def _prepare(features, nodes, anc):
    """Host-side sharding: returns (CA, CB, in_maps)."""
    core = anc // NSEG
    anc_local = anc - core * NSEG

    # global chunk counts (same NEFF for every core)
    maxA = 0
    maxB = 0
    per_core = []
    for c in range(N_CORES):
        m = core == c
        a_l = anc_local[m]
        n_l = nodes[m]
        per_core.append((a_l, n_l))
        w_l = a_l // P
        low = n_l < SPLIT
        ca = np.bincount(w_l[low], minlength=W).max() if low.any() else 0
        cb = np.bincount(w_l[~low], minlength=W).max() if (~low).any() else 0
        maxA = max(maxA, int(ca))
        maxB = max(maxB, int(cb))
    CA = max(1, math.ceil(maxA / P))
    CB = max(1, math.ceil(maxB / P))

    cnt = np.bincount(anc, minlength=N_OUT).astype(np.float32)
    recip_all = (1.0 / np.maximum(cnt, 1.0)).astype(np.float32)

    C = CA + CB
    iota = np.ascontiguousarray(
        np.tile(np.arange(P, dtype=np.float32)[None, None, :], (P, C, 1)))

    in_maps = []
    for c in range(N_CORES):
        a_l, n_l = per_core[c]
        ia, ib, sg = _prep_core(a_l, n_l, CA, CB)
        r = recip_all[c * NSEG:(c + 1) * NSEG]
        r = np.concatenate([r, np.ones(W * P - NSEG, np.float32)])
        r_sb = np.ascontiguousarray(r.reshape(W, P).T)
        in_maps.append({
            "feat": features,
            "idxa": ia,
            "idxb": ib,
            "seg": sg,
            "recip": r_sb,
            "iota": iota,
        })
    return CA, CB, in_maps


def kernel(**inputs):
    from concourse.bass_utils import run_bass_kernel_spmd

    features = np.ascontiguousarray(
        np.asarray(inputs["features"], dtype=np.float32))
    nodes = np.asarray(inputs["curr_nodes_idx"]).astype(np.int64)
    anc = np.asarray(inputs["ancestors"]).astype(np.int64)
    uall = np.asarray(inputs["uall_ancestors_idx"]).astype(np.int64)

    CA, CB, in_maps = _prepare(features, nodes, anc)

    key = (CA, CB)
    if key not in _nc_cache:
        _nc_cache[key] = _build_nc(CA, CB)
    nc = _nc_cache[key]

    res = run_bass_kernel_spmd(nc, in_maps, core_ids=list(range(N_CORES)),
                               trace=TRACE)
    global last_results
    last_results = res
    mean = np.concatenate([res.results[c]["out"] for c in range(N_CORES)],
                          axis=0)
    out = np.zeros((N_OUT, D), np.float32)
    out[uall] = mean
    return out
